# revision 1
# baseline (speedup 1.0000x reference)
"""Trainium2 Bass kernel for nn_Attention_50964081935360.

Single-query attention with a global-Frobenius-norm score scale:
  scores[b,s] = key[b,s,:] . query[b,:]
  denom      = ||key||_F  (over the WHOLE key tensor, all batches)
  p          = softmax(scores/denom) masked to s < seq_lens[b], renormalized
  out        = p[..., None] + 1e-15

Sharding: data-parallel over batch B=32 across 8 NeuronCores (4 batches per
core). The only cross-core communication is one scalar AllReduce (sum of
squares of the key shard).

Per-core streaming plan (memory-bound; key shard is 64 MiB, HBM-roofline
~185 us/core):
  for each [128, 4*1024] key super-tile (32 of them, 4 DMAs each):
    DVE : 4x affine_mul_reduce(key*q_rep, sum over d) -> 4 scores columns
    ACT : 2x activation(Square, accum_out) over 2048   -> ssq partials
          (ssq only needs a global total, so ACT overheads amortize;
          squares write to PSUM which is otherwise unused)
  The ssq -> scalar AllReduce chain runs entirely on GPSIMD/ACT-ring
  (tensor_reduce axis=XYZWC), so it fires ~30 us before the DVE stream
  drains and the collective (plus inter-core skew) hides under the stream.
  A warm-up AllReduce at kernel start pays the ncfw wakeup latency.
  Epilogue: exp(scores/denom) via ACT with per-partition scale, mask via
  iota + is_lt, renormalize via reciprocal, 32x32 DVE block-transposes so
  the output DMA is contiguous. TensorE is never used.
"""

import sys

import numpy as np

if "/opt/trn_rl_repo" not in sys.path:
    sys.path.insert(0, "/opt/trn_rl_repo")

import concourse.bacc as bacc
import concourse.bass as bass
import concourse.mybir as mybir
import concourse.tile as tile
from concourse.bass_isa import ReduceOp
from concourse.bass_utils import run_bass_kernel_spmd

B, S, D = 32, 4096, 1024
NCORES = 8
BPC = B // NCORES  # batches per core
P = 128            # s-tile partition size
NT = S // P        # s-tiles per batch (32)
NC_TILES = BPC * NT  # tiles per core (128)
PERTURB = 1e-15

F32 = mybir.dt.float32
I32 = mybir.dt.int32
ALU = mybir.AluOpType
ACTF = mybir.ActivationFunctionType

SUB = 4        # s-tiles per key super-tile
NG = NT // SUB  # super-tiles per batch (8)
KEY_BUFS = 9   # in-flight key super-tiles (2 MiB each)
NSQ = 2        # ACT square ops per super-tile (2048 elems each)


def build() -> bass.Bass:
    nc = bacc.Bacc(
        "TRN2", target_bir_lowering=False, debug=False, num_devices=NCORES
    )
    key_ext = nc.declare_dram_parameter("key", [BPC, S, D], F32, isOutput=False)
    q_ext = nc.declare_dram_parameter("query", [BPC, D], F32, isOutput=False)
    sl_ext = nc.declare_dram_parameter("seq_lens", [1, BPC], I32, isOutput=False)
    out_ext = nc.declare_dram_parameter("out", [BPC, S, 1], F32, isOutput=True)

    # Collective bounce buffers (internal DRAM; output must be Shared).
    cc_in = nc.dram_tensor("cc_in", [1, 8], F32)
    cc_out = nc.dram_tensor("cc_out", [1, 8], F32, addr_space="Shared")
    # Dummy collective buffers: a warm-up AllReduce at kernel start pays the
    # ncfw wakeup latency so the real one at the end doesn't.
    ccw_in = nc.dram_tensor("ccw_in", [1, 8], F32)
    ccw_out = nc.dram_tensor("ccw_out", [1, 8], F32, addr_space="Shared")

    key_ap = key_ext.ap()
    out_ap = out_ext.ap()

    with tile.TileContext(nc) as tc:
        with (
            tc.tile_pool(name="keys", bufs=KEY_BUFS) as kpool,
            tc.tile_pool(name="amr_scratch", bufs=4) as amrpool,
            tc.tile_pool(name="sq_psum", bufs=2, space="PSUM") as sqpool,
            tc.tile_pool(name="persist", bufs=1) as pp,
        ):
            # ---- setup: query broadcast, seq_lens, s-index ----
            # q/seq_lens ride the ACT HWDGE ring so they don't queue behind
            # the 512 KiB key loads on the sync ring.
            # q DMAs go FIRST on the sync ring (HWDGE FIFO per ring), so they
            # land before the 512 KiB key-load flood; batch 0's broadcast
            # alone gates the first AMR.
            def load_supertile(b, g):
                kt = kpool.tile([P, SUB * D], F32, tag="key")
                for j in range(SUB):
                    t = g * SUB + j
                    nc.sync.dma_start(
                        out=kt[:, j * D : (j + 1) * D],
                        in_=key_ap[b, t * P : (t + 1) * P, :],
                    )
                return kt

            q_tiles = []
            for b in range(BPC):
                qr = pp.tile([P, D], F32, tag=f"qrep{b}")
                nc.sync.dma_start(
                    out=qr[0:1, :], in_=q_ext.ap()[b : b + 1, :]
                )
                q_tiles.append(qr)
            for b in range(BPC):
                nc.gpsimd.partition_broadcast(q_tiles[b][:, :], q_tiles[b][0:1, :])
            q_rep = [q_tiles[b][:, :] for b in range(BPC)]

            # warm-up collective (result unused)
            warm = pp.tile([1, 8], F32)
            nc.vector.memset(warm[:, :], 0.0)
            nc.scalar.dma_start(out=ccw_in.ap()[:, :], in_=warm[:, :])
            nc.gpsimd.collective_compute(
                "AllReduce",
                ALU.add,
                replica_groups=[list(range(NCORES))],
                ins=[ccw_in.ap().opt()],
                outs=[ccw_out.ap().opt()],
            )

            sl_i = pp.tile([1, BPC], I32)
            nc.scalar.dma_start(out=sl_i[:, :], in_=sl_ext.ap()[:, :])
            sl_f = pp.tile([P, BPC], F32)
            nc.vector.tensor_copy(out=sl_f[0:1, :], in_=sl_i[:, :])
            nc.gpsimd.partition_broadcast(sl_f[:, :], sl_f[0:1, :])

            # s_idx[p, t] = p + 128 * t  (the sequence position of scores[p, t])
            s_idx_i = pp.tile([P, NT], I32)
            nc.gpsimd.iota(
                s_idx_i[:, :], pattern=[[P, NT]], base=0, channel_multiplier=1
            )
            s_idx = pp.tile([P, NT], F32)
            nc.vector.tensor_copy(out=s_idx[:, :], in_=s_idx_i[:, :])

            # masks depend only on s_idx/seq_lens: compute them up front so
            # the post-AllReduce tail is shorter
            masks = []
            for b in range(BPC):
                m_b = pp.tile([P, NT], F32, tag=f"m{b}")
                nc.vector.tensor_scalar(
                    out=m_b[:, :],
                    in0=s_idx[:, :],
                    scalar1=sl_f[:, b : b + 1],
                    scalar2=None,
                    op0=ALU.is_lt,
                )
                masks.append(m_b)

            # ---- main streaming loop over key super-tiles ----
            scores = pp.tile([P, NC_TILES], F32)
            ssqcols = pp.tile([P, NSQ * BPC * NG], F32)

            for b in range(BPC):
                for g in range(NG):
                    kt = load_supertile(b, g)
                    # scores columns: sum_d key*q (one DVE pass per s-tile)
                    for j in range(SUB):
                        c = b * NT + g * SUB + j
                        amr = amrpool.tile([P, D], F32, tag="amr")
                        nc.vector.affine_mul_reduce(
                            out=amr[:, :],
                            accum_out=scores[:, c : c + 1],
                            in0=kt[:, j * D : (j + 1) * D],
                            in1=q_rep[b][:, :],
                            scale=1.0,
                            bias=0.0,
                        )
                    # global ssq only needs a total: square+accum over a chunk
                    # of the super-tile per ACT op (amortizes ACT overheads);
                    # out goes to PSUM (unused otherwise, saves SBUF)
                    for h in range(NSQ):
                        c2 = NSQ * (b * NG + g) + h
                        w = SUB * D // NSQ
                        sq = sqpool.tile([P, w], F32, tag="sq")
                        nc.scalar.activation(
                            out=sq[:, :],
                            in_=kt[:, h * w : (h + 1) * w],
                            func=ACTF.Square,
                            accum_out=ssqcols[:, c2 : c2 + 1],
                        )

            # ---- local ssq reduction -> scalar, then AllReduce ----
            # Everything here stays OFF the DVE queue: the ACT squares finish
            # well before the DVE AMR stream, so a gpsimd-only reduction lets
            # the AllReduce run hidden under the stream's tail (absorbing the
            # inter-core skew wait).
            ssq_vec = pp.tile([1, 8], F32)
            nc.gpsimd.memset(ssq_vec[:, :], 0.0)
            nc.gpsimd.tensor_reduce(
                out=ssq_vec[:, 0:1], in_=ssqcols[:, :],
                axis=mybir.AxisListType.XYZWC, op=ALU.add,
            )

            nc.scalar.dma_start(out=cc_in.ap()[:, :], in_=ssq_vec[:, :])
            nc.gpsimd.collective_compute(
                "AllReduce",
                ALU.add,
                replica_groups=[list(range(NCORES))],
                ins=[cc_in.ap().opt()],
                outs=[cc_out.ap().opt()],
            )
            gssq = pp.tile([P, 1], F32)
            nc.scalar.dma_start(out=gssq[0:1, :], in_=cc_out.ap()[:, 0:1])
            nc.gpsimd.partition_broadcast(gssq[:, :], gssq[0:1, :])

            # inv = 1/sqrt(gssq) = exp(-0.5 * ln(gssq)); Ln and Exp share an
            # ACT table set (natural_log_exp_and_others), unlike Sqrt.
            lng = pp.tile([P, 1], F32)
            nc.scalar.activation(out=lng[:, :], in_=gssq[:, :], func=ACTF.Ln)
            inv_rep = pp.tile([P, 1], F32)
            nc.scalar.activation(
                out=inv_rep[:, :], in_=lng[:, :], func=ACTF.Exp, scale=-0.5
            )

            # ---- epilogue: masked softmax per batch ----
            zcols = pp.tile([P, BPC], F32)
            em = []
            for b in range(BPC):
                e_b = pp.tile([P, NT], F32, tag=f"e{b}")
                nc.scalar.activation(
                    out=e_b[:, :],
                    in_=scores[:, b * NT : (b + 1) * NT],
                    func=ACTF.Exp,
                    scale=inv_rep[:, :],
                )
                em_b = pp.tile([P, NT], F32, tag=f"em{b}")
                nc.vector.affine_mul_reduce(
                    out=em_b[:, :],
                    accum_out=zcols[:, b : b + 1],
                    in0=e_b[:, :],
                    in1=masks[b][:, :],
                    scale=1.0,
                    bias=0.0,
                )
                em.append(em_b)

            nc.gpsimd.partition_all_reduce(
                zcols[:, :], zcols[:, :], P, ReduceOp.add
            )
            invz = pp.tile([P, BPC], F32)
            nc.vector.reciprocal(out=invz[:, :], in_=zcols[:, :])

            for b in range(BPC):
                o_b = pp.tile([P, NT], F32, tag=f"o{b}")
                nc.vector.tensor_scalar(
                    out=o_b[:, :],
                    in0=em[b][:, :],
                    scalar1=invz[:, b : b + 1],
                    scalar2=PERTURB,
                    op0=ALU.mult,
                    op1=ALU.add,
                )
                # transpose [128, 32] -> [32, 128] in 32x32 DVE blocks so the
                # output DMA is contiguous (s = p + 128*t -> row t, col p)
                tr_b = pp.tile([NT, P], F32, tag=f"tr{b}")
                for a in range(P // 32):
                    nc.vector.transpose(
                        out=tr_b[0:32, a * 32 : (a + 1) * 32],
                        in_=o_b[a * 32 : (a + 1) * 32, 0:NT],
                    )
                dst = out_ap[b, :, 0].rearrange("(t p) -> t p", p=P)
                nc.sync.dma_start(out=dst, in_=tr_b[:, :])

    nc.compile()
    return nc


_NC_CACHE = None


def _get_nc():
    global _NC_CACHE
    if _NC_CACHE is None:
        _NC_CACHE = build()
    return _NC_CACHE


def make_in_maps(key, query, seq_lens):
    key = np.ascontiguousarray(np.asarray(key, dtype=np.float32))
    query = np.ascontiguousarray(np.asarray(query, dtype=np.float32))
    seq_lens = np.ascontiguousarray(np.asarray(seq_lens, dtype=np.int32))
    in_maps = []
    for c in range(NCORES):
        lo, hi = c * BPC, (c + 1) * BPC
        in_maps.append(
            {
                "key": key[lo:hi],
                "query": query[lo:hi],
                "seq_lens": seq_lens[lo:hi].reshape(1, BPC),
            }
        )
    return in_maps


def kernel(key, query, seq_lens, **run_kwargs):
    nc = _get_nc()
    in_maps = make_in_maps(key, query, seq_lens)
    res = run_bass_kernel_spmd(
        nc, in_maps, core_ids=list(range(NCORES)), **run_kwargs
    )
    outs = [res.results[c]["out"].reshape(BPC, S, 1) for c in range(NCORES)]
    full = np.concatenate(outs, axis=0).astype(np.float32)
    if run_kwargs:
        kernel.last_results = res  # expose profile info to test harness
    return full

